# revision 3
# baseline (speedup 1.0000x reference)
"""LIF spiking-neuron kernel for Trainium2 (Bass/Tile), 8-core data-parallel.

Reference semantics (per element, scan over T=8):
    mem = mem * 0.5 + x_t
    s_t = (mem > 1.0) ? 1.0 : 0.0
    mem = mem - s_t

Design (measured ~83us vs 103us f32-store baseline; DVE-bound):
  * Spikes are stored as uint8 via a single ACT op per step:
    Sign(m - 1) -> u8 cast. Sign yields {-1,0,1}; the cast maps 1 -> 1
    while -1/0 map to not-1 under either saturate or wrap semantics, so
    the host-side (u8 == 1) decode is exact. Store traffic drops 4x,
    taking DMA (21MB @ ~330GB/s ~ 63us) below the DVE wall.
  * The recurrence itself runs entirely on DVE as two fused
    scalar_tensor_tensor passes per step (~62us busy, the hard floor:
    DVE two-tensor ops run 1 elem/cycle/partition; ACT/PE cannot add two
    full tensors, DMA cannot write PSUM, and fp32 PE matmul is 4x slow):
        m        = (neg_mem * -0.5) + x_t    # DVE stt: mult, add
        s_u8     = Sign(m - 1) -> uint8      # ACT (bias AP = -1)
        neg_mem' = (m > 1.0) - m             # DVE stt: is_gt, subtract
  * Pipeline head/tail shrink: t=0 is processed in 1024-wide pieces and
    t=T-1 in 256-wide pieces (loads, stt, Sign, stores) so the pipeline
    fills early and drains fast; middle steps stay 2048-wide. Loads on
    the sync ring; stores on the scalar ring (last-step stores alternate
    scalar/gpsimd). Deep tile pools decouple ACT from the DVE chain.

Sharding: batch dim B=32 (dim 1 after temporal expand) split across 8
cores, 4 per core; per-core tensor is [T=8, 128 partitions, 4096 free]
fp32. The scan carries only per-core state; no cross-core communication.
"""

import numpy as np

import concourse.bass as bass
import concourse.bacc as bacc
import concourse.tile as tile
from concourse import mybir
from concourse.bass_utils import run_bass_kernel_spmd

T = 8
B = 32
C = 128
H = 32
W = 32
NCORES = 8
BL = B // NCORES
N = BL * C * H * W            # 524288 elements per timestep per core
P = 128
FREE = N // P                 # 4096
FCHUNK = 2048
NCH = FREE // FCHUNK          # 2
FPIECE = 1024
QP = FCHUNK // FPIECE         # pieces per chunk (t=0 fill)
FP7 = 512
QP7 = FCHUNK // FP7           # pieces per chunk (t=T-1 drain)

_ALU = mybir.AluOpType
F32 = mybir.dt.float32
U8 = mybir.dt.uint8


def build_bass():
    nc = bacc.Bacc("TRN2", target_bir_lowering=False, debug=False,
                   num_devices=NCORES)
    _F = mybir.ActivationFunctionType
    x_ap = nc.dram_tensor("x", [T, P, FREE], F32, kind="ExternalInput").ap()
    o_ap = nc.dram_tensor("out", [T, P, FREE], U8, kind="ExternalOutput").ap()

    with tile.TileContext(nc) as tc:
        with (
            tc.tile_pool(name="xp", bufs=8) as xp,
            tc.tile_pool(name="sp", bufs=8) as sp,
            tc.tile_pool(name="mp", bufs=6) as mp,
            tc.tile_pool(name="cp", bufs=1) as cp,
        ):
            neg1 = cp.tile([P, 1], F32, tag="neg1")
            nc.gpsimd.memset(neg1[:], -1.0)
            neg_mem = [None] * NCH

            # ---- t = 0: fine-grained fill ----
            # x0 pieces load into one wide tile per chunk; nm0 written
            # piecewise into a wide tile as pieces land.
            x0 = [None] * NCH
            for ci in range(NCH):
                x0[ci] = xp.tile([P, FCHUNK], F32, tag="x", name=f"x0_{ci}")
                nm = mp.tile([P, FCHUNK], F32, tag="nm", name=f"nm0_{ci}")
                for q in range(QP):
                    pq = bass.ts(q, FPIECE)
                    gq = bass.ts(ci * QP + q, FPIECE)
                    nc.sync.dma_start(x0[ci][:, pq], x_ap[0, :, gq])
                    s = sp.tile([P, FPIECE], U8, tag="s", name="s0")
                    nc.scalar.activation(s[:], x0[ci][:, pq], _F.Sign,
                                         bias=neg1)
                    nc.vector.scalar_tensor_tensor(
                        nm[:, pq], x0[ci][:, pq], 1.0, x0[ci][:, pq],
                        _ALU.is_gt, _ALU.subtract)
                    nc.scalar.dma_start(o_ap[0, :, gq], s[:])
                neg_mem[ci] = nm

            # ---- t = 1 .. T-2: wide ----
            for t in range(1, T - 1):
                for ci in range(NCH):
                    sl = bass.ts(ci, FCHUNK)
                    xt = xp.tile([P, FCHUNK], F32, tag="x")
                    nc.sync.dma_start(xt[:], x_ap[t, :, sl])
                    m = mp.tile([P, FCHUNK], F32, tag="m")
                    nc.vector.scalar_tensor_tensor(
                        m[:], neg_mem[ci][:], -0.5, xt[:],
                        _ALU.mult, _ALU.add)
                    s = sp.tile([P, FCHUNK], U8, tag="s")
                    nc.scalar.activation(s[:], m[:], _F.Sign, bias=neg1)
                    nm = mp.tile([P, FCHUNK], F32, tag="nm")
                    nc.vector.scalar_tensor_tensor(
                        nm[:], m[:], 1.0, m[:],
                        _ALU.is_gt, _ALU.subtract)
                    neg_mem[ci] = nm
                    nc.scalar.dma_start(o_ap[t, :, sl], s[:])

            # ---- t = T-1: fine-grained drain ----
            t = T - 1
            for ci in range(NCH):
                xt = xp.tile([P, FCHUNK], F32, tag="x", name=f"x7_{ci}")
                for q in range(QP7):
                    pq = bass.ts(q, FP7)
                    gq = bass.ts(ci * QP7 + q, FP7)
                    nc.sync.dma_start(xt[:, pq], x_ap[t, :, gq])
                    m = mp.tile([P, FP7], F32, tag="m7", name="m7")
                    nc.vector.scalar_tensor_tensor(
                        m[:], neg_mem[ci][:, pq], -0.5, xt[:, pq],
                        _ALU.mult, _ALU.add)
                    s = sp.tile([P, FP7], U8, tag="s", name="s7")
                    nc.scalar.activation(s[:], m[:], _F.Sign, bias=neg1)
                    stq = nc.scalar if q % 2 == 0 else nc.gpsimd
                    stq.dma_start(o_ap[t, :, gq], s[:])
    nc.compile()
    return nc


_NC_CACHE: dict = {}


def _get_nc():
    if "nc" not in _NC_CACHE:
        _NC_CACHE["nc"] = build_bass()
    return _NC_CACHE["nc"]


def kernel(x: np.ndarray) -> np.ndarray:
    x = np.asarray(x)
    assert x.shape == (T * B, C, H, W), x.shape
    in_dtype = x.dtype
    xs = x.reshape(T, B, C, H, W)

    in_maps = []
    for i in range(NCORES):
        xi = np.ascontiguousarray(xs[:, i * BL:(i + 1) * BL])
        in_maps.append({"x": xi.reshape(T, P, FREE)})

    nc = _get_nc()
    res = run_bass_kernel_spmd(nc, in_maps, list(range(NCORES)))

    out = np.empty((T, B, C, H, W), dtype=np.float32)
    for i in range(NCORES):
        u8 = res.results[i]["out"]
        out[:, i * BL:(i + 1) * BL] = (u8 == 1).astype(np.float32).reshape(
            T, BL, C, H, W)
    return out.reshape(T * B, C, H, W).astype(in_dtype, copy=False)


# revision 5
# speedup vs baseline: 1.0331x; 1.0331x over previous
"""LIF spiking-neuron kernel v7 for Trainium2 (Bass/Tile), 8-core data-parallel.

Reference semantics (per element, scan over T=8):
    mem = mem * 0.5 + x_t
    s_t = (mem > 1.0) ? 1.0 : 0.0
    mem = mem - s_t

v7 = v2/v5 (uint8 spike stores, all-DVE recurrence) + head/tail shrink:
  * t=0 and t=T-1 are processed in 512-wide pieces (loads, stt, Sign,
    stores) so the pipeline fills within ~2us and drains within ~3us,
    instead of waiting for full 2048-wide transfers;
  * middle steps stay 2048-wide (amortized instruction overhead);
  * loads on the sync ring, stores on the scalar ring, deep pools.

Per-step ops (DVE is the ~62us bottleneck; DMA ~63us):
    m        = (neg_mem * -0.5) + x_t    # DVE stt
    s_u8     = Sign(m - 1) -> uint8      # ACT; host decodes ==1
    neg_mem' = (m > 1.0) - m             # DVE stt
"""

import numpy as np

import concourse.bass as bass
import concourse.bacc as bacc
import concourse.tile as tile
from concourse import mybir
from concourse.bass_utils import run_bass_kernel_spmd

T = 8
B = 32
C = 128
H = 32
W = 32
NCORES = 8
BL = B // NCORES
N = BL * C * H * W            # 524288 elements per timestep per core
P = 128
FREE = N // P                 # 4096
FCHUNK = 2048
NCH = FREE // FCHUNK          # 2
FPIECE = 1024
QP = FCHUNK // FPIECE         # pieces per chunk (t=0 fill)
FP7 = 512
QP7 = FCHUNK // FP7           # pieces per chunk (t=T-1 drain)

_ALU = mybir.AluOpType
F32 = mybir.dt.float32
U8 = mybir.dt.uint8


def build_bass():
    nc = bacc.Bacc("TRN2", target_bir_lowering=False, debug=False,
                   num_devices=NCORES)
    _F = mybir.ActivationFunctionType
    x_ap = nc.dram_tensor("x", [T, P, FREE], F32, kind="ExternalInput").ap()
    o_ap = nc.dram_tensor("out", [T, P, FREE], U8, kind="ExternalOutput").ap()

    with tile.TileContext(nc) as tc:
        with (
            tc.tile_pool(name="xp", bufs=8) as xp,
            tc.tile_pool(name="sp", bufs=8) as sp,
            tc.tile_pool(name="mp", bufs=6) as mp,
            tc.tile_pool(name="cp", bufs=1) as cp,
        ):
            neg1 = cp.tile([P, 1], F32, tag="neg1")
            nc.gpsimd.memset(neg1[:], -1.0)
            neg_mem = [None] * NCH

            # ---- t = 0: fine-grained fill ----
            # x0 pieces load into one wide tile per chunk; nm0 written
            # piecewise into a wide tile as pieces land.
            x0 = [None] * NCH
            for ci in range(NCH):
                x0[ci] = xp.tile([P, FCHUNK], F32, tag="x", name=f"x0_{ci}")
                nm = mp.tile([P, FCHUNK], F32, tag="nm", name=f"nm0_{ci}")
                for q in range(QP):
                    pq = bass.ts(q, FPIECE)
                    gq = bass.ts(ci * QP + q, FPIECE)
                    nc.sync.dma_start(x0[ci][:, pq], x_ap[0, :, gq])
                    s = sp.tile([P, FPIECE], U8, tag="s", name="s0")
                    nc.scalar.activation(s[:], x0[ci][:, pq], _F.Sign,
                                         bias=neg1)
                    nc.vector.scalar_tensor_tensor(
                        nm[:, pq], x0[ci][:, pq], 1.0, x0[ci][:, pq],
                        _ALU.is_gt, _ALU.subtract)
                    nc.scalar.dma_start(o_ap[0, :, gq], s[:])
                neg_mem[ci] = nm

            # ---- t = 1 .. T-2: wide ----
            for t in range(1, T - 1):
                for ci in range(NCH):
                    sl = bass.ts(ci, FCHUNK)
                    xt = xp.tile([P, FCHUNK], F32, tag="x")
                    nc.sync.dma_start(xt[:], x_ap[t, :, sl])
                    m = mp.tile([P, FCHUNK], F32, tag="m")
                    nc.vector.scalar_tensor_tensor(
                        m[:], neg_mem[ci][:], -0.5, xt[:],
                        _ALU.mult, _ALU.add)
                    s = sp.tile([P, FCHUNK], U8, tag="s")
                    nc.scalar.activation(s[:], m[:], _F.Sign, bias=neg1)
                    nm = mp.tile([P, FCHUNK], F32, tag="nm")
                    nc.vector.scalar_tensor_tensor(
                        nm[:], m[:], 1.0, m[:],
                        _ALU.is_gt, _ALU.subtract)
                    neg_mem[ci] = nm
                    nc.scalar.dma_start(o_ap[t, :, sl], s[:])

            # ---- t = T-1: fine-grained drain ----
            t = T - 1
            for ci in range(NCH):
                xt = xp.tile([P, FCHUNK], F32, tag="x", name=f"x7_{ci}")
                for q in range(QP7):
                    pq = bass.ts(q, FP7)
                    gq = bass.ts(ci * QP7 + q, FP7)
                    nc.sync.dma_start(xt[:, pq], x_ap[t, :, gq])
                    m = mp.tile([P, FP7], F32, tag="m7", name="m7")
                    nc.vector.scalar_tensor_tensor(
                        m[:], neg_mem[ci][:, pq], -0.5, xt[:, pq],
                        _ALU.mult, _ALU.add)
                    s = sp.tile([P, FP7], U8, tag="s", name="s7")
                    nc.scalar.activation(s[:], m[:], _F.Sign, bias=neg1)
                    stq = nc.scalar if q % 2 == 0 else nc.gpsimd
                    stq.dma_start(o_ap[t, :, gq], s[:])
    nc.compile()
    return nc


_NC_CACHE: dict = {}


def _get_nc():
    if "nc" not in _NC_CACHE:
        _NC_CACHE["nc"] = build_bass()
    return _NC_CACHE["nc"]


def kernel(x: np.ndarray) -> np.ndarray:
    x = np.asarray(x)
    assert x.shape == (T * B, C, H, W), x.shape
    in_dtype = x.dtype
    xs = x.reshape(T, B, C, H, W)

    in_maps = []
    for i in range(NCORES):
        xi = np.ascontiguousarray(xs[:, i * BL:(i + 1) * BL])
        in_maps.append({"x": xi.reshape(T, P, FREE)})

    nc = _get_nc()
    res = run_bass_kernel_spmd(nc, in_maps, list(range(NCORES)))

    out = np.empty((T, B, C, H, W), dtype=np.float32)
    for i in range(NCORES):
        u8 = res.results[i]["out"]
        out[:, i * BL:(i + 1) * BL] = (u8 == 1).astype(np.float32).reshape(
            T, BL, C, H, W)
    return out.reshape(T * B, C, H, W).astype(in_dtype, copy=False)
